# revision 5
# baseline (speedup 1.0000x reference)
"""LocalWindowAttention (3x3 windows, B=16, 96x96, C=256, 4 heads) on 8
Trainium2 NeuronCores via Bass/Tile. Pure data parallel: 2 images per core.

v2: host pre-permutes x to bf16 window-contiguous (col-major within strips)
split by channel half, so each 7-strip round loads with 2 xbar transpose-DMAs
straight into channel-major SBUF. Attention runs on uniform 14-window
(126-token) tiles; output is written bf16 in one DMA per round and
un-permuted on the host.
"""

import numpy as np
import ml_dtypes

import concourse.bass as bass
import concourse.bacc as bacc
import concourse.tile as tile
from concourse import mybir
from concourse.bass_utils import run_bass_kernel_spmd

F32 = mybir.dt.float32
BF16 = mybir.dt.bfloat16

B = 16
NCORES = 8
IMG = B // NCORES          # images per core
C = 256
NH = 4
HD = 64
WS = 3
GRID = 96
NSTRIP = 32                # window-rows per image
NT = GRID * GRID           # tokens per image
SCALE = HD ** -0.5

# rounds per image: (start_strip, n_strips). 7-strip rounds hold exactly 16
# 14-window tiles; the 4-strip tail holds 9 full tiles + one 2-window tile.
ROUNDS = [(0, 7), (7, 7), (14, 7), (21, 7), (28, 4)]


def _tiles_for(nstrips):
    nw = nstrips * 32          # windows in round
    full, rem = divmod(nw, 14)
    t = [14] * full
    if rem:
        t.append(rem)
    return t


def _build(nc, img=IMG, reps=1):
    x = nc.declare_dram_parameter("x", [img, 2, NT, 128], BF16, isOutput=False).ap()
    wqkvT = nc.declare_dram_parameter("wqkvT", [128, 2, 768], BF16, isOutput=False).ap()
    wprojT = nc.declare_dram_parameter("wprojT", [128, 2, 256], BF16, isOutput=False).ap()
    maskc = nc.declare_dram_parameter("maskc", [128, 128], BF16, isOutput=False).ap()
    onesc = nc.declare_dram_parameter("onesc", [128, 64], BF16, isOutput=False).ap()
    y = nc.declare_dram_parameter("y", [img, NT, C], BF16, isOutput=True).ap()

    with tile.TileContext(nc) as tc:
        with (
            tc.tile_pool(name="const", bufs=1) as constp,
            tc.tile_pool(name="sb", bufs=1) as sb,
            tc.tile_pool(name="ps", bufs=1, space="PSUM") as ps,
        ):
            wq_sb = constp.tile([128, 2, 768], BF16)
            nc.sync.dma_start(out=wq_sb[:], in_=wqkvT[:])
            wp_sb = constp.tile([128, 2, 256], BF16)
            nc.sync.dma_start(out=wp_sb[:], in_=wprojT[:])
            mask_sb = constp.tile([128, 128], BF16)
            nc.sync.dma_start(out=mask_sb[:], in_=maskc[:])
            ones_sb = constp.tile([128, 64], BF16)
            nc.sync.dma_start(out=ones_sb[:], in_=onesc[:])

            def _all():
                for b_ in range(img):
                    for (s0, ns) in ROUNDS:
                        _round(nc, sb, ps, x, y, b_, s0, ns,
                               wq_sb, wp_sb, mask_sb, ones_sb)

            if reps == 1:
                _all()
            else:
                with tc.For_i(0, reps, 1):
                    _all()
    return nc


def _round(nc, sb, ps, x, y, img, s0, ns, wq_sb, wp_sb, mask_sb, ones_sb):
    nt = ns * 288                  # tokens this round
    t0 = s0 * 288
    tiles = _tiles_for(ns)         # window counts per tile
    T = len(tiles)
    # token offsets per tile
    offs = np.cumsum([0] + [9 * w for w in tiles]).tolist()

    # chunks of <=4 tiles for qk / denom / ao (504-token granularity)
    chunks = []
    for c0 in range(0, T, 4):
        tl = list(range(c0, min(c0 + 4, T)))
        chunks.append((offs[tl[0]], tl))

    # ---- load xT channel-major via xbar transpose DMA (one per kc half) ----
    xT = sb.tile([128, 2, 2048], BF16, tag="xT", bufs=2)
    for cc in range(2):
        nc.sync.dma_start(out=xT[:, cc, 0:nt], in_=x[img, cc, t0:t0 + nt, :],
                          transpose=True)
    pad = min(nt + 128, 2048)
    nc.vector.memset(xT[:, :, nt:pad], 0.0)

    # ---- q^T, k^T channel-major; chunk mc holds heads (2mc, 2mc+1) ----
    qT = sb.tile([128, 2, 2048], BF16, tag="qT", bufs=2)
    kT = sb.tile([128, 2, 2048], BF16, tag="kT", bufs=2)
    nc.vector.memset(kT[:, :, nt:pad], 0.0)
    def _evac(eng, out, in_):
        if eng is nc.scalar:
            eng.copy(out=out, in_=in_)
        else:
            eng.tensor_copy(out=out, in_=in_)

    for t_, base, dst, eng in ((0, 0, qT, nc.vector), (1, 256, kT, nc.vector)):
        for mc in range(2):
            for (f0, tl) in chunks:
                nc_ = offs[tl[-1] + 1] - f0
                qp = ps.tile([128, 512], F32, tag="qk", bufs=2)
                for kc in range(2):
                    nc.tensor.matmul(
                        out=qp[:, 0:nc_],
                        lhsT=wq_sb[:, kc, base + 128 * mc: base + 128 * mc + 128],
                        rhs=xT[:, kc, f0:f0 + nc_],
                        start=(kc == 0),
                        stop=(kc == 1),
                    )
                _evac(eng, dst[:, mc, f0:f0 + nc_], qp[:, 0:nc_])

    # ---- v token-major per tile: [kb, 256] ----
    v_sb = sb.tile([126, 16, 256], BF16, tag="vs", bufs=2)
    for tp_ in range(0, T, 2):
        pair = [t for t in (tp_, tp_ + 1) if t < T]
        vp = ps.tile([128, 2, 256], F32, tag="sm", bufs=2)
        for j, t_ in enumerate(pair):
            f0 = offs[t_]
            for kc in range(2):
                nc.tensor.matmul(
                    out=vp[:, j, 0:256],
                    lhsT=xT[:, kc, f0:f0 + 128],
                    rhs=wq_sb[:, kc, 512:768],
                    start=(kc == 0),
                    stop=(kc == 1),
                )
        if len(pair) == 2 and tiles[pair[1]] == 14:
            nc.scalar.copy(out=v_sb[0:126, tp_:tp_ + 2, :], in_=vp[0:126, :, :])
        else:
            for j, t_ in enumerate(pair):
                kb = 9 * tiles[t_]
                nc.scalar.copy(out=v_sb[0:kb, t_, :], in_=vp[0:kb, j, :])

    # ---- QK^T logits per tile; exp -> expm [k, tile, (hh,mc), q] ----
    # row-group hh writes its own PSUM bank (free-offset 512*hh): mixing row
    # groups within one bank is an unrecoverable HW fault.
    expm = sb.tile([126, 16, 4, 126], BF16, tag="expm", bufs=2)
    if tiles[-1] != 14:
        nc.vector.memset(expm[:, T - 1, :, :], 0.0)
    for t_ in range(T):
        kb = 9 * tiles[t_]
        f0 = offs[t_]
        aL = ps.tile([128, 2, 512], F32, tag="att", bufs=2)
        for mc in range(2):
            for hh in range(2):
                p0 = 64 * hh
                nc.tensor.matmul(
                    out=aL[:, hh, 126 * mc: 126 * mc + kb],
                    lhsT=kT[p0:p0 + 64, mc, f0:f0 + 128],
                    rhs=qT[p0:p0 + 64, mc, f0:f0 + kb],
                    start=True,
                    stop=True,
                )
        ein = bass.AP(tensor=aL.tensor, offset=aL.offset,
                      ap=[[aL.ap[0][0], kb], [512, 2], [126, 2], [1, kb]])
        nc.scalar.activation(
            out=expm[0:kb, t_, 0:4, 0:kb], in_=ein,
            func=mybir.ActivationFunctionType.Exp, scale=SCALE)

    # ---- mask: expm *= blockdiag(9), one op over all tiles ----
    m = mask_sb[0:126, 0:126]
    Th_ = T // 2
    for lo, hi, eng in ((0, Th_, nc.vector), (Th_, T, nc.vector)):
        mb = bass.AP(tensor=m.tensor, offset=m.offset,
                     ap=[m.ap[0], [0, hi - lo], [0, 4], m.ap[1]])
        eng.tensor_mul(
            out=expm[:, lo:hi, :, :], in0=expm[:, lo:hi, :, :], in1=mb)

    # ---- denominators broadcast over 64-row groups via ones-matmul ----
    rbc = sb.tile([128, 2, 2048], F32, tag="rbc", bufs=2)
    for Th in range(2):
        dps = []
        for (f0, tl) in chunks:
            dp = ps.tile([128, 512], F32, tag="sm", bufs=2)
            for hh in range(2):
                h = 2 * Th + hh
                hc = 2 * (h % 2) + h // 2
                # per-tile rhs extents (tail tile is shorter)
                full = [t for t in tl if tiles[t] == 14]
                if full:
                    e0 = expm[0:126, full[0], hc, 0:126]
                    rhs = bass.AP(tensor=e0.tensor, offset=e0.offset,
                                  ap=[e0.ap[0], [4 * 126, len(full)],
                                      [1, 126]])
                    nc.tensor.matmul(
                        out=dp[64 * hh:64 * hh + 64,
                               0:126 * len(full)],
                        lhsT=ones_sb[0:126, 0:64],
                        rhs=rhs,
                        start=True, stop=True,
                        tile_position=(0, 64 * hh),
                    )
                for t in tl:
                    if tiles[t] == 14:
                        continue
                    kb = 9 * tiles[t]
                    nc.tensor.matmul(
                        out=dp[64 * hh:64 * hh + 64,
                               offs[t] - f0: offs[t] - f0 + kb],
                        lhsT=ones_sb[0:126, 0:64],
                        rhs=expm[0:126, t, hc, 0:kb],
                        start=True, stop=True,
                        tile_position=(0, 64 * hh),
                    )
            dps.append((f0, tl, dp))
        for (f0, tl, dp) in dps:
            nc_ = offs[tl[-1] + 1] - f0
            nc.vector.reciprocal_approx_fast(
                out=rbc[:, Th, f0:f0 + nc_], in_=dp[:, 0:nc_])

    # ---- AV: unnormalized channel-major ao; normalize during evac ----
    ao = sb.tile([128, 2, 2048], BF16, tag="ao", bufs=2)
    for Th in range(2):
        for (f0, tl) in chunks:
            nc_ = offs[tl[-1] + 1] - f0
            ap_ = ps.tile([128, 512], F32, tag="sm", bufs=2)
            for t in tl:
                kb = 9 * tiles[t]
                for hh in range(2):
                    h = 2 * Th + hh
                    hc = 2 * (h % 2) + h // 2
                    nc.tensor.matmul(
                        out=ap_[64 * hh:64 * hh + 64,
                                offs[t] - f0: offs[t] - f0 + kb],
                        lhsT=v_sb[0:kb, t, 64 * h: 64 * h + 64],
                        rhs=expm[0:kb, t, hc, 0:kb],
                        start=True, stop=True,
                        tile_position=(0, 64 * hh),
                    )
            nc.vector.tensor_mul(out=ao[:, Th, f0:f0 + nc_],
                                 in0=ap_[:, 0:nc_],
                                 in1=rbc[:, Th, f0:f0 + nc_])

    # ---- proj per 96-token chunk + evac to strip-major out_sb ----
    out_sb = sb.tile([96, 21, 256], BF16, tag="outs", bufs=2)
    ng = nt // 96
    for gp in range(0, ng, 2):
        pair = [g for g in (gp, gp + 1) if g < ng]
        op = ps.tile([128, 2, 256], F32, tag="sm", bufs=2)
        for j, g in enumerate(pair):
            for Th in range(2):
                nc.tensor.matmul(
                    out=op[0:96, j, 0:256],
                    lhsT=ao[:, Th, 96 * g: 96 * g + 96],
                    rhs=wp_sb[:, Th, :],
                    start=(Th == 0),
                    stop=(Th == 1),
                )
        nc.vector.tensor_copy(out=out_sb[0:96, gp:gp + len(pair), :],
                              in_=op[0:96, 0:len(pair), :])

    # ---- one output DMA for the whole round ----
    yout = bass.AP(tensor=y.tensor, offset=(img * NT + t0) * C,
                   ap=[[C, 96], [96 * C, 3 * ns], [1, C]])
    nc.sync.dma_start(out=yout, in_=out_sb[0:96, 0:3 * ns, 0:256])


def _make_consts():
    bf16 = ml_dtypes.bfloat16
    mask = np.zeros((128, 128), np.float32)
    for p in range(126):
        for q in range(126):
            if p // 9 == q // 9:
                mask[p, q] = 1.0
    return {
        "maskc": mask.astype(bf16),
        "onesc": np.ones((128, 64), np.float32).astype(bf16),
    }


_NC_CACHE = {}


def _get_nc():
    if "nc" not in _NC_CACHE:
        nc = bacc.Bacc("TRN2", target_bir_lowering=False, debug=False,
                       num_devices=NCORES)
        _build(nc)
        nc.compile()
        _NC_CACHE["nc"] = nc
    return _NC_CACHE["nc"]


def _perm_x(x):
    """[B, 9216, 256] f32 raster -> [B, 2, 9216, 128] bf16 window-contiguous
    (col-major within each 3-row strip), split by channel half."""
    bf16 = ml_dtypes.bfloat16
    x = np.asarray(x, np.float32).reshape(B, NSTRIP, 3, GRID, C)
    x = x.transpose(0, 1, 3, 2, 4).reshape(B, NT, C)       # col-major tokens
    x = x.reshape(B, NT, 2, 128).transpose(0, 2, 1, 3)     # ch-half major
    return np.ascontiguousarray(x).astype(bf16)


def _unperm_y(y):
    """[img, 9216, 256] bf16 col-major tokens -> [img, 9216, 256] f32 raster."""
    y = np.asarray(y, np.float32).reshape(-1, NSTRIP, GRID, 3, C)
    y = y.transpose(0, 1, 3, 2, 4).reshape(-1, NT, C)
    return y


def _in_maps(x, Wqkv, Wproj):
    bf16 = ml_dtypes.bfloat16
    consts = _make_consts()
    consts["wqkvT"] = np.ascontiguousarray(
        np.asarray(Wqkv, np.float32).T.reshape(2, 128, 768).transpose(1, 0, 2)
    ).astype(bf16)
    consts["wprojT"] = np.ascontiguousarray(
        np.asarray(Wproj, np.float32).T.reshape(2, 128, 256).transpose(1, 0, 2)
    ).astype(bf16)
    xp = _perm_x(x)
    return [{"x": xp[IMG * c: IMG * c + IMG], **consts} for c in range(NCORES)]


def kernel(x, Wqkv, Wproj, H, W):
    assert int(H) == GRID and int(W) == GRID
    nc = _get_nc()
    res = run_bass_kernel_spmd(nc, _in_maps(x, Wqkv, Wproj), list(range(NCORES)))
    out = np.concatenate([_unperm_y(res.results[c]["y"]) for c in range(NCORES)],
                         axis=0)
    return np.ascontiguousarray(out.reshape(B, NT, C)).astype(np.float32)


# revision 7
# speedup vs baseline: 1.2270x; 1.2270x over previous
"""LocalWindowAttention (3x3 windows, B=16, 96x96, C=256, 4 heads) on 8
Trainium2 NeuronCores via Bass/Tile. Pure data parallel: 2 images per core.

v2: host pre-permutes x to bf16 window-contiguous (col-major within strips)
split by channel half, so each 7-strip round loads with 2 xbar transpose-DMAs
straight into channel-major SBUF. Attention runs on uniform 14-window
(126-token) tiles; output is written bf16 in one DMA per round and
un-permuted on the host.
"""

import numpy as np
import ml_dtypes

import concourse.bass as bass
import concourse.bacc as bacc
import concourse.tile as tile
from concourse import mybir
from concourse.bass_utils import run_bass_kernel_spmd

F32 = mybir.dt.float32
BF16 = mybir.dt.bfloat16

B = 16
NCORES = 8
IMG = B // NCORES          # images per core
C = 256
NH = 4
HD = 64
WS = 3
GRID = 96
NSTRIP = 32                # window-rows per image
NT = GRID * GRID           # tokens per image
SCALE = HD ** -0.5

# rounds per image: (start_strip, n_strips). 7-strip rounds hold exactly 16
# 14-window tiles; the 4-strip tail holds 9 full tiles + one 2-window tile.
ROUNDS = [(0, 7), (7, 7), (14, 7), (21, 7), (28, 4)]


def _tiles_for(nstrips):
    nw = nstrips * 32          # windows in round
    full, rem = divmod(nw, 14)
    t = [14] * full
    if rem:
        t.append(rem)
    return t


def _build(nc, img=IMG, reps=1):
    x = nc.declare_dram_parameter("x", [img, 2, NT, 128], BF16, isOutput=False).ap()
    wqkvT = nc.declare_dram_parameter("wqkvT", [128, 2, 768], BF16, isOutput=False).ap()
    wprojT = nc.declare_dram_parameter("wprojT", [128, 2, 256], BF16, isOutput=False).ap()
    maskc = nc.declare_dram_parameter("maskc", [128, 128], BF16, isOutput=False).ap()
    onesc = nc.declare_dram_parameter("onesc", [128, 64], BF16, isOutput=False).ap()
    y = nc.declare_dram_parameter("y", [img, NT, C], BF16, isOutput=True).ap()

    with tile.TileContext(nc) as tc:
        with (
            tc.tile_pool(name="const", bufs=1) as constp,
            tc.tile_pool(name="sb", bufs=1) as sb,
            tc.tile_pool(name="ps", bufs=1, space="PSUM") as ps,
        ):
            wq_sb = constp.tile([128, 2, 768], BF16)
            nc.sync.dma_start(out=wq_sb[:], in_=wqkvT[:])
            wp_sb = constp.tile([128, 2, 256], BF16)
            nc.sync.dma_start(out=wp_sb[:], in_=wprojT[:])
            mask_sb = constp.tile([128, 128], BF16)
            nc.sync.dma_start(out=mask_sb[:], in_=maskc[:])
            ones_sb = constp.tile([128, 64], BF16)
            nc.sync.dma_start(out=ones_sb[:], in_=onesc[:])

            def _all():
                for b_ in range(img):
                    for (s0, ns) in ROUNDS:
                        _round(nc, sb, ps, x, y, b_, s0, ns,
                               wq_sb, wp_sb, mask_sb, ones_sb)

            if reps == 1:
                _all()
            else:
                with tc.For_i(0, reps, 1):
                    _all()
    return nc


def _round(nc, sb, ps, x, y, img, s0, ns, wq_sb, wp_sb, mask_sb, ones_sb):
    nt = ns * 288                  # tokens this round
    t0 = s0 * 288
    tiles = _tiles_for(ns)         # window counts per tile
    T = len(tiles)
    # token offsets per tile
    offs = np.cumsum([0] + [9 * w for w in tiles]).tolist()

    # chunks of <=4 tiles for qk / denom / ao (504-token granularity)
    chunks = []
    for c0 in range(0, T, 4):
        tl = list(range(c0, min(c0 + 4, T)))
        chunks.append((offs[tl[0]], tl))

    # ---- load xT channel-major via xbar transpose DMA (one per kc half) ----
    xT = sb.tile([128, 2, 2048], BF16, tag="xT", bufs=2)
    for cc in range(2):
        nc.sync.dma_start(out=xT[:, cc, 0:nt], in_=x[img, cc, t0:t0 + nt, :],
                          transpose=True)
    pad = min(nt + 128, 2048)
    nc.vector.memset(xT[:, :, nt:pad], 0.0)

    # ---- q^T, k^T channel-major; chunk mc holds heads (2mc, 2mc+1) ----
    qT = sb.tile([128, 2, 2048], BF16, tag="qT", bufs=2)
    kT = sb.tile([128, 2, 2048], BF16, tag="kT", bufs=2)
    nc.vector.memset(kT[:, :, nt:pad], 0.0)
    def _evac(eng, out, in_):
        if eng is nc.scalar:
            eng.copy(out=out, in_=in_)
        else:
            eng.tensor_copy(out=out, in_=in_)

    for t_, base, dst, eng in ((0, 0, qT, nc.vector), (1, 256, kT, nc.scalar)):
        for mc in range(2):
            for (f0, tl) in chunks:
                nc_ = offs[tl[-1] + 1] - f0
                qp = ps.tile([128, 512], F32, tag="qk", bufs=2)
                for kc in range(2):
                    nc.tensor.matmul(
                        out=qp[:, 0:nc_],
                        lhsT=wq_sb[:, kc, base + 128 * mc: base + 128 * mc + 128],
                        rhs=xT[:, kc, f0:f0 + nc_],
                        start=(kc == 0),
                        stop=(kc == 1),
                    )
                _evac(eng, dst[:, mc, f0:f0 + nc_], qp[:, 0:nc_])

    # ---- v token-major per tile: [kb, 256] ----
    v_sb = sb.tile([126, 16, 256], BF16, tag="vs", bufs=2)
    for tp_ in range(0, T, 2):
        pair = [t for t in (tp_, tp_ + 1) if t < T]
        vp = ps.tile([128, 2, 256], F32, tag="sm", bufs=2)
        for j, t_ in enumerate(pair):
            f0 = offs[t_]
            for kc in range(2):
                nc.tensor.matmul(
                    out=vp[:, j, 0:256],
                    lhsT=xT[:, kc, f0:f0 + 128],
                    rhs=wq_sb[:, kc, 512:768],
                    start=(kc == 0),
                    stop=(kc == 1),
                )
        eng = nc.scalar if (tp_ // 2) % 2 == 0 else nc.vector
        if len(pair) == 2 and tiles[pair[1]] == 14:
            _evac(eng, v_sb[0:126, tp_:tp_ + 2, :], vp[0:126, :, :])
        else:
            for j, t_ in enumerate(pair):
                kb = 9 * tiles[t_]
                _evac(eng, v_sb[0:kb, t_, :], vp[0:kb, j, :])

    # ---- QK^T logits per tile; exp -> expm [k, tile, (hh,mc), q] ----
    # row-group hh writes its own PSUM bank (free-offset 512*hh): mixing row
    # groups within one bank is an unrecoverable HW fault.
    expm = sb.tile([126, 16, 4, 126], BF16, tag="expm", bufs=2)
    if tiles[-1] != 14:
        nc.vector.memset(expm[:, T - 1, :, :], 0.0)
    for t_ in range(T):
        kb = 9 * tiles[t_]
        f0 = offs[t_]
        aL = ps.tile([128, 2, 512], F32, tag="att", bufs=1)
        for mc in range(2):
            for hh in range(2):
                p0 = 64 * hh
                nc.tensor.matmul(
                    out=aL[:, hh, 126 * mc: 126 * mc + kb],
                    lhsT=kT[p0:p0 + 64, mc, f0:f0 + 128],
                    rhs=qT[p0:p0 + 64, mc, f0:f0 + kb],
                    start=True,
                    stop=True,
                )
        ein = bass.AP(tensor=aL.tensor, offset=aL.offset,
                      ap=[[aL.ap[0][0], kb], [512, 2], [126, 2], [1, kb]])
        nc.scalar.activation(
            out=expm[0:kb, t_, 0:4, 0:kb], in_=ein,
            func=mybir.ActivationFunctionType.Exp, scale=SCALE)

    # ---- mask: expm *= blockdiag(9), one op over all tiles ----
    m = mask_sb[0:126, 0:126]
    Th_ = T // 2
    for lo, hi, eng in ((0, Th_, nc.vector), (Th_, T, nc.vector)):
        mb = bass.AP(tensor=m.tensor, offset=m.offset,
                     ap=[m.ap[0], [0, hi - lo], [0, 4], m.ap[1]])
        eng.tensor_mul(
            out=expm[:, lo:hi, :, :], in0=expm[:, lo:hi, :, :], in1=mb)

    # ---- denominators (ones-matmul) + AV + normalize-by-divide, per chunk ----
    ao = sb.tile([128, 2, 2048], BF16, tag="ao", bufs=2)
    nc.vector.memset(ao[:, :, nt:pad], 0.0)
    for Th in range(2):
        for (f0, tl) in chunks:
            nc_ = offs[tl[-1] + 1] - f0
            dp = ps.tile([128, 512], F32, tag="dp", bufs=2)
            for hh in range(2):
                h = 2 * Th + hh
                hc = 2 * (h % 2) + h // 2
                full = [t for t in tl if tiles[t] == 14]
                if full:
                    e0 = expm[0:126, full[0], hc, 0:126]
                    rhs = bass.AP(tensor=e0.tensor, offset=e0.offset,
                                  ap=[e0.ap[0], [4 * 126, len(full)],
                                      [1, 126]])
                    nc.tensor.matmul(
                        out=dp[64 * hh:64 * hh + 64,
                               0:126 * len(full)],
                        lhsT=ones_sb[0:126, 0:64],
                        rhs=rhs,
                        start=True, stop=True,
                        tile_position=(0, 64 * hh),
                    )
                for t in tl:
                    if tiles[t] == 14:
                        continue
                    kb = 9 * tiles[t]
                    nc.tensor.matmul(
                        out=dp[64 * hh:64 * hh + 64,
                               offs[t] - f0: offs[t] - f0 + kb],
                        lhsT=ones_sb[0:126, 0:64],
                        rhs=expm[0:126, t, hc, 0:kb],
                        start=True, stop=True,
                        tile_position=(0, 64 * hh),
                    )
            ap_ = ps.tile([128, 512], F32, tag="sm", bufs=2)
            for t in tl:
                kb = 9 * tiles[t]
                for hh in range(2):
                    h = 2 * Th + hh
                    hc = 2 * (h % 2) + h // 2
                    nc.tensor.matmul(
                        out=ap_[64 * hh:64 * hh + 64,
                                offs[t] - f0: offs[t] - f0 + kb],
                        lhsT=v_sb[0:kb, t, 64 * h: 64 * h + 64],
                        rhs=expm[0:kb, t, hc, 0:kb],
                        start=True, stop=True,
                        tile_position=(0, 64 * hh),
                    )
            rbc = sb.tile([128, 512], F32, tag="rbc", bufs=2)
            nc.vector.reciprocal_approx_fast(
                out=rbc[:, 0:nc_], in_=dp[:, 0:nc_])
            nc.vector.tensor_mul(out=ao[:, Th, f0:f0 + nc_],
                                 in0=ap_[:, 0:nc_],
                                 in1=rbc[:, 0:nc_])

    # ---- proj per 96-token chunk + evac to strip-major out_sb ----
    out_sb = sb.tile([126, 16, 256], BF16, tag="outs", bufs=2)
    for tp_ in range(0, T, 2):
        pair = [t for t in (tp_, tp_ + 1) if t < T]
        op = ps.tile([128, 2, 256], F32, tag="sm", bufs=2)
        for j, t_ in enumerate(pair):
            kb = 9 * tiles[t_]
            f0 = offs[t_]
            for Th in range(2):
                nc.tensor.matmul(
                    out=op[:, j, 0:256],
                    lhsT=ao[:, Th, f0:f0 + 128],
                    rhs=wp_sb[:, Th, :],
                    start=(Th == 0),
                    stop=(Th == 1),
                )
        eng = nc.vector if (tp_ // 2) % 2 == 0 else nc.scalar
        if len(pair) == 2 and tiles[pair[1]] == 14:
            if eng is nc.scalar:
                eng.copy(out=out_sb[0:126, tp_:tp_ + 2, :], in_=op[0:126, :, :])
            else:
                eng.tensor_copy(out=out_sb[0:126, tp_:tp_ + 2, :], in_=op[0:126, :, :])
        else:
            for j, t_ in enumerate(pair):
                kb = 9 * tiles[t_]
                if eng is nc.scalar:
                    eng.copy(out=out_sb[0:kb, t_, :], in_=op[0:kb, j, :])
                else:
                    eng.tensor_copy(out=out_sb[0:kb, t_, :], in_=op[0:kb, j, :])

    # ---- output DMA(s): token index = 126*t + p (full tiles only) ----
    fullT = sum(1 for w in tiles if w == 14)
    yout = bass.AP(tensor=y.tensor, offset=(img * NT + t0) * C,
                   ap=[[C, 126], [126 * C, fullT], [1, C]])
    nc.sync.dma_start(out=yout, in_=out_sb[0:126, 0:fullT, 0:256])
    if fullT != T:
        kb = 9 * tiles[-1]
        ytail = bass.AP(tensor=y.tensor,
                        offset=(img * NT + t0 + 126 * fullT) * C,
                        ap=[[C, kb], [1, C]])
        nc.sync.dma_start(out=ytail, in_=out_sb[0:kb, T - 1, 0:256])


def _make_consts():
    bf16 = ml_dtypes.bfloat16
    mask = np.zeros((128, 128), np.float32)
    for p in range(126):
        for q in range(126):
            if p // 9 == q // 9:
                mask[p, q] = 1.0
    return {
        "maskc": mask.astype(bf16),
        "onesc": np.ones((128, 64), np.float32).astype(bf16),
    }


_NC_CACHE = {}


def _get_nc():
    if "nc" not in _NC_CACHE:
        nc = bacc.Bacc("TRN2", target_bir_lowering=False, debug=False,
                       num_devices=NCORES)
        _build(nc)
        nc.compile()
        _NC_CACHE["nc"] = nc
    return _NC_CACHE["nc"]


def _perm_x(x):
    """[B, 9216, 256] f32 raster -> [B, 2, 9216, 128] bf16 window-contiguous
    (col-major within each 3-row strip), split by channel half."""
    bf16 = ml_dtypes.bfloat16
    x = np.asarray(x, np.float32).reshape(B, NSTRIP, 3, GRID, C)
    x = x.transpose(0, 1, 3, 2, 4).reshape(B, NT, C)       # col-major tokens
    x = x.reshape(B, NT, 2, 128).transpose(0, 2, 1, 3)     # ch-half major
    return np.ascontiguousarray(x).astype(bf16)


def _unperm_y(y):
    """[img, 9216, 256] bf16 col-major tokens -> [img, 9216, 256] f32 raster."""
    y = np.asarray(y, np.float32).reshape(-1, NSTRIP, GRID, 3, C)
    y = y.transpose(0, 1, 3, 2, 4).reshape(-1, NT, C)
    return y


def _in_maps(x, Wqkv, Wproj):
    bf16 = ml_dtypes.bfloat16
    consts = _make_consts()
    consts["wqkvT"] = np.ascontiguousarray(
        np.asarray(Wqkv, np.float32).T.reshape(2, 128, 768).transpose(1, 0, 2)
    ).astype(bf16)
    consts["wprojT"] = np.ascontiguousarray(
        np.asarray(Wproj, np.float32).T.reshape(2, 128, 256).transpose(1, 0, 2)
    ).astype(bf16)
    xp = _perm_x(x)
    return [{"x": xp[IMG * c: IMG * c + IMG], **consts} for c in range(NCORES)]


def kernel(x, Wqkv, Wproj, H, W):
    assert int(H) == GRID and int(W) == GRID
    nc = _get_nc()
    res = run_bass_kernel_spmd(nc, _in_maps(x, Wqkv, Wproj), list(range(NCORES)))
    out = np.concatenate([_unperm_y(res.results[c]["y"]) for c in range(NCORES)],
                         axis=0)
    return np.ascontiguousarray(out.reshape(B, NT, C)).astype(np.float32)
